# revision 30
# baseline (speedup 1.0000x reference)
"""CenterLoss kernel for Trainium2 (8 NeuronCores, Bass).

Math: the reference builds the full [B, C] squared-distance matrix, masks it
to one column per row (the label), clips ALL entries to [1e-12, 1e12], sums
and divides by B.  Because the mask keeps exactly one entry per row:

    loss = ( sum_b clip(||x_b - centers[l_b]||^2, 1e-12, 1e12)
             + (B*C - B) * 1e-12 ) / B

so the kernel is a row gather of `centers` plus an elementwise reduction --
no GEMM needed.  For this distribution every ||x_b - c_{l_b}||^2 is
~4096 +- 450 (the minimum over the batch is ~3650), so the [1e-12, 1e12]
clip is mathematically the identity on the per-row distances; the kernel
therefore reduces to the batch total on-device.

Sharding: data-parallel over the batch.  Each of the 8 cores receives 512
rows of x (fp8e4m3, flat [128, 4*2048]: partition p holds batch rows
4p..4p+3), labels wrapped [128, 4] int32 (lab[p, k] = labels[4p + k]),
and the full centers table (fp8e4m3).  fp8 on the wire quarters HBM
traffic (quantization bias ~0.2%, far inside the 2e-2 gate).

Mixed SBUF precision, tuned to the two measured bottlenecks (SBUF-fabric
S2M bytes gate DMA completions; fp8 DVE ops lock GPSIMD's descriptor
generator out of SBUF): blocks 0, 1 upcast to bf16 in the SDMA datapath
so their DVE subtracts run early in fast non-contending 2x mode; blocks
2, 3 stay fp8 in SBUF (halving their S2M bytes so the late pairs land
sooner) and their 1x subtracts are scheduled after descriptor generation
has finished.  Ring layout: labels then x2, x3 on the sync HWDGE ring;
x0, x1 (cast) and the four gathers ride the SWDGE ring in FIFO
need-order, with gather descriptor generation running back-to-back.

Row sums of squares: ACT Square+accumulate (blocks 0..2 and the front of
3), DVE mult+reduce (the back of 3).  The final reduction runs
on-device: PE ones-matmul folds partitions into PSUM, DVE folds the
remaining row, and the scalar engine stores the single f32 via a
register write (a dummy early store warms the store path).  The host
sums the 8 per-core scalars and adds the clip constant.

Hand-placed semaphores (no TileContext) to minimize scheduling overhead.
"""

import numpy as np
import ml_dtypes
from contextlib import ExitStack

import concourse.bacc as bacc
import concourse.bass as bass
import concourse.mybir as mybir
from concourse.bass_utils import run_bass_kernel_spmd

B = 4096
D = 2048
C = 8192
N_CORES = 8
SHARD = B // N_CORES          # 512
P = 128
T = SHARD // P                # 4
SP = 1280                     # ACT/DVE split point for block 3

FP8 = ml_dtypes.float8_e4m3

_nc_cache = {}


def _build(scratch=131072):
    key = (scratch,)
    if key in _nc_cache:
        return _nc_cache[key]

    nc = bacc.Bacc("TRN2", target_bir_lowering=False, debug=False,
                   dynamic_dma_scratch_size=scratch)
    fp8 = mybir.dt.float8e4
    bf16 = mybir.dt.bfloat16
    f32 = mybir.dt.float32
    x = nc.dram_tensor("x", [P, T * D], fp8, kind="ExternalInput")
    labels = nc.dram_tensor("labels", [P, T], mybir.dt.int32, kind="ExternalInput")
    centers = nc.dram_tensor("centers", [C, D], fp8, kind="ExternalInput")
    out = nc.dram_tensor("out", [1, 1], f32, kind="ExternalOutput")

    NCOL = 5  # rowsum columns: d0, d1, d2, 3-front, 3-back

    with ExitStack() as ctx:
        block = ctx.enter_context(nc.Block(no_gpsimd_drain=True))
        lab = ctx.enter_context(nc.sbuf_tensor("lab", [P, T], mybir.dt.int32))
        # blocks 0, 1 upcast to bf16; blocks 2, 3 stay fp8 (fewer S2M bytes)
        xb01 = ctx.enter_context(nc.sbuf_tensor("xb01", [P, 2 * D], bf16))
        xb23 = ctx.enter_context(nc.sbuf_tensor("xb23", [P, 2 * D], fp8))
        gt01 = [ctx.enter_context(nc.sbuf_tensor(f"gt{t}", [P, D], bf16)) for t in (0, 1)]
        gt23 = [ctx.enter_context(nc.sbuf_tensor(f"gt{t}", [P, D], fp8)) for t in (2, 3)]
        # bf16 differences for blocks 2, 3 (written by the 1x fp8 subtracts)
        ds23 = ctx.enter_context(nc.sbuf_tensor("ds23", [P, 2 * D], bf16))
        sq3b = ctx.enter_context(nc.sbuf_tensor("sq3b", [P, D - SP], bf16))
        rowsum = ctx.enter_context(nc.sbuf_tensor("rowsum", [P, NCOL], f32))
        ones = ctx.enter_context(nc.sbuf_tensor("ones", [P, 1], f32))
        final = ctx.enter_context(nc.sbuf_tensor("final", [1, 1], f32))
        colsum = ctx.enter_context(nc.psum_tensor("colsum", [1, NCOL], f32))

        s_lab = ctx.enter_context(nc.semaphore("s_lab"))
        s_x = [ctx.enter_context(nc.semaphore(f"s_x{t}")) for t in range(T)]
        s_g = [ctx.enter_context(nc.semaphore(f"s_g{t}")) for t in range(T)]
        s_sub = [ctx.enter_context(nc.semaphore(f"s_sub{t}")) for t in range(T)]
        s_sub3b = ctx.enter_context(nc.semaphore("s_sub3b"))
        s_mul = ctx.enter_context(nc.semaphore("s_mul"))
        s_acc = ctx.enter_context(nc.semaphore("s_acc"))
        s_ones = ctx.enter_context(nc.semaphore("s_ones"))
        s_mm = ctx.enter_context(nc.semaphore("s_mm"))
        s_red = ctx.enter_context(nc.semaphore("s_red"))

        def xblk(t):
            return xb01[:, t * D:(t + 1) * D] if t < 2 else \
                xb23[:, (t - 2) * D:(t - 1) * D]

        def gt(t):
            return gt01[t] if t < 2 else gt23[t - 2]

        @block.sync
        def _(sync):
            sync.dma_start(out=lab[:, :], in_=labels[:, :]).then_inc(s_lab, 16)

        @block.gpsimd
        def _(gpsimd):
            # SWDGE ring in FIFO need-order; with x2/x3 off-ring the four
            # gather descriptor generations run back-to-back
            for t in (0, 1):
                gpsimd.dma_start(out=xblk(t), in_=x[:, t * D:(t + 1) * D]
                                 ).then_inc(s_x[t], 16)
            gpsimd.memset(ones[:, :], 1.0).then_inc(s_ones, 1)
            gpsimd.wait_ge(s_lab, 16)
            # all gathers first: their descriptor generations pack
            # back-to-back (x-issues between gens would delay g2/g3 by ~2us);
            # x2/x3 ride a separate SWDGE queue and still land in time
            order = [("g", 0), ("g", 1), ("g", 2), ("g", 3), ("x", 2), ("x", 3)]
            for kind, t in order:
                if kind == "x":
                    gpsimd.dma_start(out=xblk(t), in_=x[:, t * D:(t + 1) * D]
                                     ).then_inc(s_x[t], 16)
                else:
                    gpsimd.indirect_dma_start(
                        out=gt(t)[:, :],
                        out_offset=None,
                        in_=centers[:, :],
                        in_offset=bass.IndirectOffsetOnAxis(
                            ap=lab[:, t:t + 1], axis=0),
                    ).then_inc(s_g[t], 16)

        @block.vector
        def _(vector):
            for t in (0, 1):
                vector.wait_ge(s_x[t], 16)
                vector.wait_ge(s_g[t], 16)
                # in-place bf16 subtract: 2x DVE mode, never contends with
                # GPSIMD descriptor generation
                vector.tensor_tensor(
                    out=xblk(t), in0=xblk(t), in1=gt(t)[:, :],
                    op=mybir.AluOpType.subtract,
                ).then_inc(s_sub[t], 1)
            # fp8 subtracts (1x) for blocks 2, 3 run only after descriptor
            # generation is long done, so the 2-port contention never happens
            vector.wait_ge(s_x[2], 16)
            vector.wait_ge(s_g[2], 16)
            vector.tensor_tensor(
                out=ds23[:, :D], in0=xblk(2), in1=gt(2)[:, :],
                op=mybir.AluOpType.subtract,
            ).then_inc(s_sub[2], 1)
            vector.wait_ge(s_x[3], 16)
            vector.wait_ge(s_g[3], 16)
            vector.tensor_tensor(
                out=ds23[:, D:D + SP], in0=xblk(3)[:, :SP], in1=gt(3)[:, :SP],
                op=mybir.AluOpType.subtract,
            ).then_inc(s_sub[3], 1)
            vector.tensor_tensor(
                out=ds23[:, D + SP:], in0=xblk(3)[:, SP:], in1=gt(3)[:, SP:],
                op=mybir.AluOpType.subtract,
            ).then_inc(s_sub3b, 1)
            # back of block 3 on DVE (self-wait: the subtract must retire
            # before its output is re-read); bf16 ds keeps the 2x mult
            vector.wait_ge(s_sub3b, 1)
            vector.tensor_tensor(
                out=sq3b[:, :], in0=ds23[:, D + SP:], in1=ds23[:, D + SP:],
                op=mybir.AluOpType.mult,
            ).then_inc(s_mul, 1)
            vector.wait_ge(s_mul, 1)
            vector.tensor_reduce(
                out=rowsum[:, 4:5], in_=sq3b[:, :],
                axis=mybir.AxisListType.X, op=mybir.AluOpType.add,
            ).then_inc(s_acc, 1)
            vector.wait_ge(s_mm, 1)
            vector.tensor_reduce(
                out=final[:, :], in_=colsum[:1, :],
                axis=mybir.AxisListType.X, op=mybir.AluOpType.add,
            ).then_inc(s_red, 1)

        @block.scalar
        def _(scalar):
            # dummy early store warms the scalar store path off-critical-path
            with scalar.register("gr_pre") as gr_pre:
                scalar.reg_mov(gr_pre, 0)
                scalar.reg_save(out[:1, :1].bitcast(mybir.dt.int32), gr_pre)
            for t in (0, 1):
                scalar.wait_ge(s_sub[t], 1)
                scalar.activation(
                    out=xblk(t), in_=xblk(t),
                    func=mybir.ActivationFunctionType.Square,
                    accum_out=rowsum[:, t:t + 1],
                ).then_inc(s_acc, 1)
            scalar.wait_ge(s_sub[2], 1)
            scalar.activation(
                out=ds23[:, :D], in_=ds23[:, :D],
                func=mybir.ActivationFunctionType.Square,
                accum_out=rowsum[:, 2:3],
            ).then_inc(s_acc, 1)
            scalar.wait_ge(s_sub[3], 1)
            scalar.activation(
                out=ds23[:, D:D + SP], in_=ds23[:, D:D + SP],
                func=mybir.ActivationFunctionType.Square,
                accum_out=rowsum[:, 3:4],
            ).then_inc(s_acc, 1)
            with scalar.register("gr_out") as gr_out:
                scalar.wait_ge(s_red, 1)
                scalar.reg_load(gr_out, final[:1, :1].bitcast(mybir.dt.int32))
                scalar.reg_save(out[:1, :1].bitcast(mybir.dt.int32), gr_out)

        @block.tensor
        def _(tensor):
            tensor.wait_ge(s_ones, 1)
            tensor.wait_ge(s_acc, NCOL)
            tensor.matmul(
                colsum[:1, :], ones[:, :], rowsum[:, :], start=True, stop=True,
            ).then_inc(s_mm, 1)

    nc.compile()
    _nc_cache[key] = nc
    return nc


def _make_in_maps(x, labels, centers):
    x = np.asarray(x, dtype=np.float32).astype(FP8)
    centers = np.ascontiguousarray(np.asarray(centers, dtype=np.float32).astype(FP8))
    lab32 = np.asarray(labels).astype(np.int32)
    in_maps = []
    for i in range(N_CORES):
        sl = slice(i * SHARD, (i + 1) * SHARD)
        in_maps.append({
            # partition p holds batch rows 4p..4p+3 of this shard
            "x": np.ascontiguousarray(x[sl]).reshape(P, T * D),
            # lab[p, k] = labels[4p + k], pairing with x column block k
            "labels": np.ascontiguousarray(lab32[sl].reshape(P, T)),
            "centers": centers,
        })
    return in_maps


def _finish(results):
    total = 0.0
    for r in results:
        # on-device per-core total; the clip is the identity for this data
        total += float(np.asarray(r["out"], dtype=np.float64)[0, 0])
    total += (B * C - B) * 1e-12
    return np.float32(total / B)


def kernel(x, labels, centers):
    nc = _build()
    in_maps = _make_in_maps(x, labels, centers)
    res = run_bass_kernel_spmd(nc, in_maps, core_ids=list(range(N_CORES)))
    return _finish(res.results)


# revision 31
# speedup vs baseline: 1.0380x; 1.0380x over previous
"""CenterLoss kernel for Trainium2 (8 NeuronCores, Bass).

Math: the reference builds the full [B, C] squared-distance matrix, masks it
to one column per row (the label), clips ALL entries to [1e-12, 1e12], sums
and divides by B.  Because the mask keeps exactly one entry per row:

    loss = ( sum_b clip(||x_b - centers[l_b]||^2, 1e-12, 1e12)
             + (B*C - B) * 1e-12 ) / B

so the kernel is a row gather of `centers` plus an elementwise reduction --
no GEMM needed.  For this distribution every ||x_b - c_{l_b}||^2 is
~4096 +- 450 (the minimum over the batch is ~3650), so the [1e-12, 1e12]
clip is mathematically the identity on the per-row distances; the kernel
therefore reduces to the batch total on-device.

Sharding: data-parallel over the batch.  Each of the 8 cores receives 512
rows of x (fp8e4m3, flat [128, 4*2048]: partition p holds batch rows
4p..4p+3), labels wrapped [128, 4] int32 (lab[p, k] = labels[4p + k]),
and the full centers table (fp8e4m3).  fp8 on the wire quarters HBM
traffic (quantization bias ~0.2%, far inside the 2e-2 gate).

Mixed SBUF precision, tuned to the two measured bottlenecks (SBUF-fabric
S2M bytes gate DMA completions; fp8 DVE ops lock GPSIMD's descriptor
generator out of SBUF): blocks 0, 1 upcast to bf16 in the SDMA datapath
so their DVE subtracts run early in fast non-contending 2x mode; blocks
2, 3 stay fp8 in SBUF (halving their S2M bytes so the late pairs land
sooner) and their 1x subtracts are scheduled after descriptor generation
has finished.  Ring layout: labels then x2, x3 on the sync HWDGE ring;
x0, x1 (cast) and the four gathers ride the SWDGE ring in FIFO
need-order, with gather descriptor generation running back-to-back.

Row sums of squares: ACT Square+accumulate (blocks 0..2 and the front of
3), DVE mult+reduce (the back of 3).  The final reduction runs
on-device: PE ones-matmul folds partitions into PSUM, DVE folds the
remaining row, and the scalar engine stores the single f32 via a
register write (a dummy early store warms the store path).  The host
sums the 8 per-core scalars and adds the clip constant.

Hand-placed semaphores (no TileContext) to minimize scheduling overhead.
"""

import numpy as np
import ml_dtypes
from contextlib import ExitStack

import concourse.bacc as bacc
import concourse.bass as bass
import concourse.mybir as mybir
from concourse.bass_utils import run_bass_kernel_spmd

B = 4096
D = 2048
C = 8192
N_CORES = 8
SHARD = B // N_CORES          # 512
P = 128
T = SHARD // P                # 4
SP = 1280                     # ACT/DVE split point for block 3

FP8 = ml_dtypes.float8_e4m3

_nc_cache = {}


def _build(scratch=131072):
    key = (scratch,)
    if key in _nc_cache:
        return _nc_cache[key]

    nc = bacc.Bacc("TRN2", target_bir_lowering=False, debug=False,
                   dynamic_dma_scratch_size=scratch)
    fp8 = mybir.dt.float8e4
    bf16 = mybir.dt.bfloat16
    f32 = mybir.dt.float32
    x = nc.dram_tensor("x", [P, T * D], fp8, kind="ExternalInput")
    labels = nc.dram_tensor("labels", [P, T], mybir.dt.int32, kind="ExternalInput")
    centers = nc.dram_tensor("centers", [C, D], fp8, kind="ExternalInput")
    out = nc.dram_tensor("out", [1, 1], f32, kind="ExternalOutput")

    NCOL = 5  # rowsum columns: d0, d1, d2, 3-front, 3-back

    with ExitStack() as ctx:
        block = ctx.enter_context(nc.Block(no_gpsimd_drain=True))
        lab = ctx.enter_context(nc.sbuf_tensor("lab", [P, T], mybir.dt.int32))
        # blocks 0, 1 upcast to bf16; blocks 2, 3 stay fp8 (fewer S2M bytes)
        xb01 = ctx.enter_context(nc.sbuf_tensor("xb01", [P, 2 * D], bf16))
        xb23 = ctx.enter_context(nc.sbuf_tensor("xb23", [P, 2 * D], fp8))
        gt01 = [ctx.enter_context(nc.sbuf_tensor(f"gt{t}", [P, D], bf16)) for t in (0, 1)]
        gt23 = [ctx.enter_context(nc.sbuf_tensor(f"gt{t}", [P, D], fp8)) for t in (2, 3)]
        # bf16 differences for blocks 2, 3 (written by the 1x fp8 subtracts)
        ds23 = ctx.enter_context(nc.sbuf_tensor("ds23", [P, 2 * D], bf16))
        sq3b = ctx.enter_context(nc.sbuf_tensor("sq3b", [P, D - SP], bf16))
        rowsum = ctx.enter_context(nc.sbuf_tensor("rowsum", [P, NCOL], f32))
        ones = ctx.enter_context(nc.sbuf_tensor("ones", [P, 1], f32))
        final = ctx.enter_context(nc.sbuf_tensor("final", [1, 1], f32))
        colsum = ctx.enter_context(nc.psum_tensor("colsum", [1, NCOL], f32))

        s_lab = ctx.enter_context(nc.semaphore("s_lab"))
        s_x = [ctx.enter_context(nc.semaphore(f"s_x{t}")) for t in range(T)]
        s_g = [ctx.enter_context(nc.semaphore(f"s_g{t}")) for t in range(T)]
        s_sub = [ctx.enter_context(nc.semaphore(f"s_sub{t}")) for t in range(T)]
        s_sub3b = ctx.enter_context(nc.semaphore("s_sub3b"))
        s_mul = ctx.enter_context(nc.semaphore("s_mul"))
        s_acc = ctx.enter_context(nc.semaphore("s_acc"))
        s_ones = ctx.enter_context(nc.semaphore("s_ones"))
        s_mm = ctx.enter_context(nc.semaphore("s_mm"))
        s_red = ctx.enter_context(nc.semaphore("s_red"))

        def xblk(t):
            return xb01[:, t * D:(t + 1) * D] if t < 2 else \
                xb23[:, (t - 2) * D:(t - 1) * D]

        def gt(t):
            return gt01[t] if t < 2 else gt23[t - 2]

        @block.sync
        def _(sync):
            sync.dma_start(out=lab[:, :], in_=labels[:, :]).then_inc(s_lab, 16)

        @block.gpsimd
        def _(gpsimd):
            # SWDGE ring in FIFO need-order; with x2/x3 off-ring the four
            # gather descriptor generations run back-to-back
            for t in (0, 1):
                gpsimd.dma_start(out=xblk(t), in_=x[:, t * D:(t + 1) * D]
                                 ).then_inc(s_x[t], 16)
            gpsimd.memset(ones[:, :], 1.0).then_inc(s_ones, 1)
            gpsimd.wait_ge(s_lab, 16)
            # g0, g1 first; the fp8 x2/x3 chunks slot in before g2/g3 so the
            # late pairs land just in time without competing with early pairs
            order = [("g", 0), ("g", 1), ("x", 2), ("x", 3), ("g", 2), ("g", 3)]
            for kind, t in order:
                if kind == "x":
                    gpsimd.dma_start(out=xblk(t), in_=x[:, t * D:(t + 1) * D]
                                     ).then_inc(s_x[t], 16)
                else:
                    gpsimd.indirect_dma_start(
                        out=gt(t)[:, :],
                        out_offset=None,
                        in_=centers[:, :],
                        in_offset=bass.IndirectOffsetOnAxis(
                            ap=lab[:, t:t + 1], axis=0),
                    ).then_inc(s_g[t], 16)

        @block.vector
        def _(vector):
            for t in (0, 1):
                vector.wait_ge(s_x[t], 16)
                vector.wait_ge(s_g[t], 16)
                # in-place bf16 subtract: 2x DVE mode, never contends with
                # GPSIMD descriptor generation
                vector.tensor_tensor(
                    out=xblk(t), in0=xblk(t), in1=gt(t)[:, :],
                    op=mybir.AluOpType.subtract,
                ).then_inc(s_sub[t], 1)
            # fp8 subtracts (1x) for blocks 2, 3 run only after descriptor
            # generation is long done, so the 2-port contention never happens
            vector.wait_ge(s_x[2], 16)
            vector.wait_ge(s_g[2], 16)
            vector.tensor_tensor(
                out=ds23[:, :D], in0=xblk(2), in1=gt(2)[:, :],
                op=mybir.AluOpType.subtract,
            ).then_inc(s_sub[2], 1)
            vector.wait_ge(s_x[3], 16)
            vector.wait_ge(s_g[3], 16)
            vector.tensor_tensor(
                out=ds23[:, D:D + SP], in0=xblk(3)[:, :SP], in1=gt(3)[:, :SP],
                op=mybir.AluOpType.subtract,
            ).then_inc(s_sub[3], 1)
            vector.tensor_tensor(
                out=ds23[:, D + SP:], in0=xblk(3)[:, SP:], in1=gt(3)[:, SP:],
                op=mybir.AluOpType.subtract,
            ).then_inc(s_sub3b, 1)
            # back of block 3 on DVE (self-wait: the subtract must retire
            # before its output is re-read); bf16 ds keeps the 2x mult
            vector.wait_ge(s_sub3b, 1)
            vector.tensor_tensor(
                out=sq3b[:, :], in0=ds23[:, D + SP:], in1=ds23[:, D + SP:],
                op=mybir.AluOpType.mult,
            ).then_inc(s_mul, 1)
            vector.wait_ge(s_mul, 1)
            vector.tensor_reduce(
                out=rowsum[:, 4:5], in_=sq3b[:, :],
                axis=mybir.AxisListType.X, op=mybir.AluOpType.add,
            ).then_inc(s_acc, 1)
            vector.wait_ge(s_mm, 1)
            vector.tensor_reduce(
                out=final[:, :], in_=colsum[:1, :],
                axis=mybir.AxisListType.X, op=mybir.AluOpType.add,
            ).then_inc(s_red, 1)

        @block.scalar
        def _(scalar):
            # dummy early store warms the scalar store path off-critical-path
            with scalar.register("gr_pre") as gr_pre:
                scalar.reg_mov(gr_pre, 0)
                scalar.reg_save(out[:1, :1].bitcast(mybir.dt.int32), gr_pre)
            for t in (0, 1):
                scalar.wait_ge(s_sub[t], 1)
                scalar.activation(
                    out=xblk(t), in_=xblk(t),
                    func=mybir.ActivationFunctionType.Square,
                    accum_out=rowsum[:, t:t + 1],
                ).then_inc(s_acc, 1)
            scalar.wait_ge(s_sub[2], 1)
            scalar.activation(
                out=ds23[:, :D], in_=ds23[:, :D],
                func=mybir.ActivationFunctionType.Square,
                accum_out=rowsum[:, 2:3],
            ).then_inc(s_acc, 1)
            scalar.wait_ge(s_sub[3], 1)
            scalar.activation(
                out=ds23[:, D:D + SP], in_=ds23[:, D:D + SP],
                func=mybir.ActivationFunctionType.Square,
                accum_out=rowsum[:, 3:4],
            ).then_inc(s_acc, 1)
            with scalar.register("gr_out") as gr_out:
                scalar.wait_ge(s_red, 1)
                scalar.reg_load(gr_out, final[:1, :1].bitcast(mybir.dt.int32))
                scalar.reg_save(out[:1, :1].bitcast(mybir.dt.int32), gr_out)

        @block.tensor
        def _(tensor):
            tensor.wait_ge(s_ones, 1)
            tensor.wait_ge(s_acc, NCOL)
            tensor.matmul(
                colsum[:1, :], ones[:, :], rowsum[:, :], start=True, stop=True,
            ).then_inc(s_mm, 1)

    nc.compile()
    _nc_cache[key] = nc
    return nc


def _make_in_maps(x, labels, centers):
    x = np.asarray(x, dtype=np.float32).astype(FP8)
    centers = np.ascontiguousarray(np.asarray(centers, dtype=np.float32).astype(FP8))
    lab32 = np.asarray(labels).astype(np.int32)
    in_maps = []
    for i in range(N_CORES):
        sl = slice(i * SHARD, (i + 1) * SHARD)
        in_maps.append({
            # partition p holds batch rows 4p..4p+3 of this shard
            "x": np.ascontiguousarray(x[sl]).reshape(P, T * D),
            # lab[p, k] = labels[4p + k], pairing with x column block k
            "labels": np.ascontiguousarray(lab32[sl].reshape(P, T)),
            "centers": centers,
        })
    return in_maps


def _finish(results):
    total = 0.0
    for r in results:
        # on-device per-core total; the clip is the identity for this data
        total += float(np.asarray(r["out"], dtype=np.float64)[0, 0])
    total += (B * C - B) * 1e-12
    return np.float32(total / B)


def kernel(x, labels, centers):
    nc = _build()
    in_maps = _make_in_maps(x, labels, centers)
    res = run_bass_kernel_spmd(nc, in_maps, core_ids=list(range(N_CORES)))
    return _finish(res.results)
